# revision 6
# baseline (speedup 1.0000x reference)
"""KV-cache ring-buffer update + rolling re-linearization, on 8 trn2 NeuronCores.

Problem semantics (nn_KVCache):
  k_cache[:, pos] = k ; v_cache[:, pos] = v          (scatter into ring buffer)
  out = concat([cache[:, split:], cache[:, :split]]) (roll to logical order)

For the given inputs (pos = arange(7680..8703) % 8192, max_pos = 8703) the
whole computation reduces to contiguous row copies:
  out[:, 0:7168]    = cache[:, 512:7680]   (old data, 28 MiB per tensor/batch)
  out[:, 7168:8192] = new[:, 0:1024]       ( 4 MiB per tensor/batch)

Sharding: pure batch-parallel (B=8 -> 1 batch per core), no communication.
Each core runs a handful of DRAM->DRAM HWDGE DMA copies split across the
two HWDGE rings (sync + scalar); the 16 SDMA engines are the roofline.
"""

import os

import numpy as np

B, S_NEW, H, D = 8, 1024, 16, 128
MAX_SIZE = 8192
HD = H * D  # 2048 fp16 elements = 4096 B per row

N_CORES = 8

# Tuning knobs (env-overridable for experiments; defaults = best known).
VARIANT = int(os.environ.get("KV_VARIANT", "2"))
MAXSEM = int(os.environ.get("KV_MAXSEM", "0"))


def _patch_walrus_maxsem(maxsem):
    """Append --max-sem-num to the walrus arg list for this process."""
    import concourse.bass_utils as BU

    if getattr(BU, "_kv_maxsem_patched", None) == maxsem:
        return
    orig = BU.get_walrus_args.__wrapped__ if hasattr(BU.get_walrus_args, "__wrapped__") else BU.get_walrus_args

    def wrapped(*a, **kw):
        return [*orig(*a, **kw), f"--max-sem-num={maxsem}"]

    wrapped.__wrapped__ = orig
    BU.get_walrus_args = wrapped
    BU._kv_maxsem_patched = maxsem


def _copy_plan(pos, max_pos):
    """Derive the list of contiguous row-copies implied by (pos, max_pos).

    Returns (out_rows, runs) with runs = [(dst_row, src: 'new'|'cache',
    src_row, n_rows), ...] such that
      out[dst:dst+n] = (k|v)[src_row:src_row+n]        if src == 'new'
      out[dst:dst+n] = (k|v)_cache[src_row:src_row+n]  if src == 'cache'
    """
    pos = (np.asarray(pos).astype(np.int64) % MAX_SIZE).ravel()
    next_pos = int(max_pos) + 1
    if next_pos > MAX_SIZE:
        out_rows = MAX_SIZE
        split = next_pos % MAX_SIZE
        order = (np.arange(MAX_SIZE, dtype=np.int64) + split) % MAX_SIZE
    else:
        out_rows = next_pos
        order = np.arange(next_pos, dtype=np.int64)
    newpos = np.full(MAX_SIZE, -1, dtype=np.int64)
    newpos[pos] = np.arange(pos.shape[0], dtype=np.int64)  # duplicate pos: last wins
    sel = newpos[order]
    is_new = sel >= 0
    src_row = np.where(is_new, sel, order)
    runs = []
    j = 0
    while j < out_rows:
        s = j
        while (
            j + 1 < out_rows
            and is_new[j + 1] == is_new[s]
            and src_row[j + 1] == src_row[s] + (j + 1 - s)
        ):
            j += 1
        runs.append((s, "new" if is_new[s] else "cache", int(src_row[s]), j - s + 1))
        j += 1
    return out_rows, runs


def _strip_const_memsets(nc):
    """Drop the framework's const-AP MEMSETs (nothing in this kernel reads
    them; their start timestamp otherwise pins first_useful_time)."""
    for func in nc.m.functions:
        for blk in func.blocks:
            blk.instructions = [
                i
                for i in blk.instructions
                if not (
                    type(i).__name__ == "InstMemset"
                    and any(
                        getattr(o, "name", "").startswith("const-")
                        for o in (i.outs or [])
                    )
                )
            ]


def _build(out_rows, runs):
    import concourse.bass as bass
    import concourse.mybir as mybir

    nc = bass.Bass(
        enable_partition_id=False,
        monotonic_sem_count=0,
        detect_race_conditions=False,
    )
    f16 = mybir.dt.float16
    kc = nc.declare_dram_parameter("kc", [MAX_SIZE, HD], f16, isOutput=False)
    vc = nc.declare_dram_parameter("vc", [MAX_SIZE, HD], f16, isOutput=False)
    kn = nc.declare_dram_parameter("kn", [S_NEW, HD], f16, isOutput=False)
    vn = nc.declare_dram_parameter("vn", [S_NEW, HD], f16, isOutput=False)
    ko = nc.declare_dram_parameter("ko", [out_rows, HD], f16, isOutput=True)
    vo = nc.declare_dram_parameter("vo", [out_rows, HD], f16, isOutput=True)

    # k copies issue from the sync HWDGE ring, v copies from the scalar
    # HWDGE ring: descriptor generation overlaps and both rings keep the 16
    # SDMA engines fed from the first microsecond. Each issuing engine
    # waits for its own ring's completion semaphore — the program must not
    # retire before the output DMAs have landed (the epilogue Drain does
    # NOT drain HWDGE rings).
    k_sem = nc.alloc_semaphore("k_sem")
    v_sem = nc.alloc_semaphore("v_sem")
    for dst, src, row, n in runs:
        sk = kn if src == "new" else kc
        nc.sync.dma_start(out=ko[dst : dst + n], in_=sk[row : row + n]).then_inc(
            k_sem, 16
        )
    for dst, src, row, n in runs:
        sv = vn if src == "new" else vc
        nc.scalar.dma_start(out=vo[dst : dst + n], in_=sv[row : row + n]).then_inc(
            v_sem, 16
        )
    nc.sync.wait_ge(k_sem, 16 * len(runs))
    nc.scalar.wait_ge(v_sem, 16 * len(runs))

    if VARIANT >= 2:
        _strip_const_memsets(nc)
    return nc


def _run(k, v, k_cache, v_cache, pos, max_pos, trace=False):
    from concourse.bass_utils import run_bass_kernel_spmd

    if MAXSEM:
        _patch_walrus_maxsem(MAXSEM)

    k = np.asarray(k)
    v = np.asarray(v)
    k_cache = np.asarray(k_cache)
    v_cache = np.asarray(v_cache)

    out_rows, runs = _copy_plan(pos, max_pos)
    nc = _build(out_rows, runs)

    in_maps = [
        {
            "kc": k_cache[b].reshape(MAX_SIZE, HD),
            "vc": v_cache[b].reshape(MAX_SIZE, HD),
            "kn": k[b].reshape(S_NEW, HD),
            "vn": v[b].reshape(S_NEW, HD),
        }
        for b in range(N_CORES)
    ]
    res = run_bass_kernel_spmd(nc, in_maps, list(range(N_CORES)), trace=trace)
    k_out = np.stack([r["ko"] for r in res.results]).reshape(B, out_rows, H, D)
    v_out = np.stack([r["vo"] for r in res.results]).reshape(B, out_rows, H, D)
    return (k_out, v_out), res


def kernel(k, v, k_cache, v_cache, pos, max_pos):
    (k_out, v_out), _ = _run(k, v, k_cache, v_cache, pos, max_pos)
    return k_out, v_out


# revision 7
# speedup vs baseline: 1.1883x; 1.1883x over previous
"""KV-cache ring-buffer update + rolling re-linearization, on 8 trn2 NeuronCores.

Problem semantics (nn_KVCache):
  k_cache[:, pos] = k ; v_cache[:, pos] = v          (scatter into ring buffer)
  out = concat([cache[:, split:], cache[:, :split]]) (roll to logical order)

For the given inputs (pos = arange(7680..8703) % 8192, max_pos = 8703) the
whole computation reduces to contiguous row copies:
  out[:, 0:7168]    = cache[:, 512:7680]   (old data, 28 MiB per tensor/batch)
  out[:, 7168:8192] = new[:, 0:1024]       ( 4 MiB per tensor/batch)

Sharding: pure batch-parallel (B=8 -> 1 batch per core), no communication.

Per core this is 64 MiB of DRAM->DRAM copy. k copies issue on the sync
HWDGE ring, v copies on the scalar HWDGE ring; the rings spread 64 KiB
packets round-robin over the 16 SDMA engines, which stay ~100% busy at
~21 GB/s each for the whole window — the HBM/DMA roofline for this part
(measured: ~202 us DMA window + ~2 us DGE start latency + ~8 us NEFF
postamble, ~213 us total vs 216.8 us for the previous Block-based kernel).

Structure notes (why there is no Block/barrier here):
 - No bass Block: the NEFF pre/postamble already barrier all engines, so
   the Block's extra drain + barrier round only added tail latency.
 - Each issuing engine waits on its ring's completion semaphore; the
   program must not retire before the output DMAs have landed (the
   postamble's per-engine Drain does NOT drain HWDGE rings — verified:
   without the waits results stay correct only by scheduling luck and the
   rings are still live at NEFF teardown).
 - The framework's const-AP MEMSETs are stripped: nothing reads them,
   and their early timestamp otherwise pins first_useful_time ~1.3 us
   before the first real instruction.
"""

import numpy as np

B, S_NEW, H, D = 8, 1024, 16, 128
MAX_SIZE = 8192
HD = H * D  # 2048 fp16 elements = 4096 B per row

N_CORES = 8


def _copy_plan(pos, max_pos):
    """Derive the list of contiguous row-copies implied by (pos, max_pos).

    Returns (out_rows, runs) with runs = [(dst_row, src: 'new'|'cache',
    src_row, n_rows), ...] such that
      out[dst:dst+n] = (k|v)[src_row:src_row+n]        if src == 'new'
      out[dst:dst+n] = (k|v)_cache[src_row:src_row+n]  if src == 'cache'
    """
    pos = (np.asarray(pos).astype(np.int64) % MAX_SIZE).ravel()
    next_pos = int(max_pos) + 1
    if next_pos > MAX_SIZE:
        out_rows = MAX_SIZE
        split = next_pos % MAX_SIZE
        order = (np.arange(MAX_SIZE, dtype=np.int64) + split) % MAX_SIZE
    else:
        out_rows = next_pos
        order = np.arange(next_pos, dtype=np.int64)
    newpos = np.full(MAX_SIZE, -1, dtype=np.int64)
    newpos[pos] = np.arange(pos.shape[0], dtype=np.int64)  # duplicate pos: last wins
    sel = newpos[order]
    is_new = sel >= 0
    src_row = np.where(is_new, sel, order)
    runs = []
    j = 0
    while j < out_rows:
        s = j
        while (
            j + 1 < out_rows
            and is_new[j + 1] == is_new[s]
            and src_row[j + 1] == src_row[s] + (j + 1 - s)
        ):
            j += 1
        runs.append((s, "new" if is_new[s] else "cache", int(src_row[s]), j - s + 1))
        j += 1
    return out_rows, runs


def _strip_const_memsets(nc):
    """Drop the framework's const-AP MEMSETs: nothing in this kernel reads
    them, and their start timestamp otherwise pins first_useful_time."""
    for func in nc.m.functions:
        for blk in func.blocks:
            blk.instructions = [
                i
                for i in blk.instructions
                if not (
                    type(i).__name__ == "InstMemset"
                    and any(
                        getattr(o, "name", "").startswith("const-")
                        for o in (i.outs or [])
                    )
                )
            ]


def _build(out_rows, runs):
    import concourse.bass as bass
    import concourse.mybir as mybir

    nc = bass.Bass(
        enable_partition_id=False,
        monotonic_sem_count=0,
        detect_race_conditions=False,
    )
    f16 = mybir.dt.float16
    kc = nc.declare_dram_parameter("kc", [MAX_SIZE, HD], f16, isOutput=False)
    vc = nc.declare_dram_parameter("vc", [MAX_SIZE, HD], f16, isOutput=False)
    kn = nc.declare_dram_parameter("kn", [S_NEW, HD], f16, isOutput=False)
    vn = nc.declare_dram_parameter("vn", [S_NEW, HD], f16, isOutput=False)
    ko = nc.declare_dram_parameter("ko", [out_rows, HD], f16, isOutput=True)
    vo = nc.declare_dram_parameter("vo", [out_rows, HD], f16, isOutput=True)

    k_sem = nc.alloc_semaphore("k_sem")
    v_sem = nc.alloc_semaphore("v_sem")
    for dst, src, row, n in runs:
        sk = kn if src == "new" else kc
        nc.sync.dma_start(out=ko[dst : dst + n], in_=sk[row : row + n]).then_inc(
            k_sem, 16
        )
    for dst, src, row, n in runs:
        sv = vn if src == "new" else vc
        nc.scalar.dma_start(out=vo[dst : dst + n], in_=sv[row : row + n]).then_inc(
            v_sem, 16
        )
    nc.sync.wait_ge(k_sem, 16 * len(runs))
    nc.scalar.wait_ge(v_sem, 16 * len(runs))

    _strip_const_memsets(nc)
    return nc


def _run(k, v, k_cache, v_cache, pos, max_pos, trace=False):
    from concourse.bass_utils import run_bass_kernel_spmd

    k = np.asarray(k)
    v = np.asarray(v)
    k_cache = np.asarray(k_cache)
    v_cache = np.asarray(v_cache)

    out_rows, runs = _copy_plan(pos, max_pos)
    nc = _build(out_rows, runs)

    in_maps = [
        {
            "kc": k_cache[b].reshape(MAX_SIZE, HD),
            "vc": v_cache[b].reshape(MAX_SIZE, HD),
            "kn": k[b].reshape(S_NEW, HD),
            "vn": v[b].reshape(S_NEW, HD),
        }
        for b in range(N_CORES)
    ]
    res = run_bass_kernel_spmd(nc, in_maps, list(range(N_CORES)), trace=trace)
    k_out = np.stack([r["ko"] for r in res.results]).reshape(B, out_rows, H, D)
    v_out = np.stack([r["vo"] for r in res.results]).reshape(B, out_rows, H, D)
    return (k_out, v_out), res


def kernel(k, v, k_cache, v_cache, pos, max_pos):
    (k_out, v_out), _ = _run(k, v, k_cache, v_cache, pos, max_pos)
    return k_out, v_out
